# revision 72
# baseline (speedup 1.0000x reference)
"""Trainium2 Bass kernel for nn_Attention_81793357185069 (v3).

4-group attention: N=16, L=M=1024, in/param dim 512, planes 512, out 2048.
Data-parallel over batch N across 8 NeuronCores (2 batches/core), zero
collectives. All matmuls bf16 with fp32 PSUM accumulation.

v2 core design (kept):
  - softmax denominator mostly off the PE: DVE running-sum over the
    exp'd score tiles (bf16), then ONE ones-matmul pair on the summed
    tile (2 MMs/unit vs 16 -- saves ~24us of PE ones-matmuls).
    (gpsimd partition_all_reduce was tried: 6.8us/call on HW, too slow.)
  - scores/filler PSUM tiles are [128,1024] 2-bank pairs: exp and PSUM
    evacuations run as single fat instructions
  - deferred normalization: the SV accumulator leaves PSUM unnormalized
    right after its last matmul; svT = svu * (1/den) later from SBUF
  - reciprocal_approx_fast (5x faster than DVE RECIPROCAL, 18-bit)
  - PSUM: 3x[128,1024] score/filler bufs + 1x[128,1024] SV accum = 8 banks

v3 changes (CoreSim no-exec 173.7us -> 161.0us; HW baseline was 189.7us):
  - output DRAM tensor is bf16 (halves out-DMA bytes; +0.3e-3 L2 err),
    out DMAs alternate SP/Pool queues, tail chunks flipped so the final
    DMAs interleave; final proj chunk split into two 512-wide PSUM tiles
    so its evac+DMA overlaps its own matmuls (shorter drain tail)
  - deferred DMAs (batch-1 x, proj weight) moved OFF the ACT queue onto
    the Pool/SWDGE queue in need order: a DMACopy parked in the ACT FIFO
    waiting on a DMA-lane sem stalled the slot-0 exp chain ~6us
  - weight DMAs split per group (unit (0,0) needs only group-0 columns
    of wq/wk: 128KB instead of 512KB on the critical path) and spread
    across SP+ACT queues in need order
  - prologue restructured: q halves + k half-0 only (s(0..3) read just
    kT[:,0:512]), each 512-wide piece in its OWN PSUM tile so evacs
    overlap the next piece's matmuls; k half-1 and cv(0,0) ride slot 0
    as fills in the SV-accumulator PSUM bank (idle until slot 1),
    giving the PE non-psp-gated work while exps pace psp recycling
  - zero q/k biases (the spec fills them with zeros) compile to a
    bias-free variant: plain copies, no qb/kb DMAs on the ACT queue
  - most PSUM evacuations moved from DVE to ACT (exps + evacs ~72%
    ACT busy): DVE keeps only the denominator add-chain + normalize,
    so the adds that gate the ones-matmul never queue behind an evac
  - batch-0 x tensors DMA'd as mch-major quarters alternating SP/Pool:
    the first q/k chunk's matmuls pipeline into the transfers (first
    real matmul at ~2.7us instead of ~4.4us) and each chunk's first
    512-col half arrives before its second
  - PE warmup trimmed 72 -> 28 matmuls, ending exactly when the first
    input half lands so the HAM busy window stays continuous into the
    real matmuls (the longer warmup's queue backlog was delaying the
    first real matmul ~1.5us)

Not taken: fp8 (any single fp8 GEMM adds >=2.5% L2 against the 1.9%
remaining error budget; DoubleRow needs e4m3/e5m2 whose matmul path is
3 mantissa bits), TP-over-groups sharding (adds collectives for zero
balance gain vs pure DP).
"""

import math

import ml_dtypes
import numpy as np

import concourse.bass as bass
import concourse.bass_isa as bass_isa
import concourse.mybir as mybir
import concourse.tile as tile
from concourse import bacc
from concourse.bass_utils import run_bass_kernel_spmd

N_CORES = 8
N = 16
B = N // N_CORES  # batches per core
T = 1024  # L == M
C = 512  # in/param dim
P = 512  # planes
O = 2048  # out dim
G = 4  # groups
D = P // G  # 128 group planes
ATTN_SCALE = P ** (-0.5)
EQ_SCALE = 1.0 / math.sqrt(C)

CT = C // 128  # 4 contraction tiles
TT = T // 128  # 8 l/m tiles
MCH = T // 512  # 2 moving chunks of 512

BF = mybir.dt.bfloat16
F32 = mybir.dt.float32

_CACHE: dict = {}


def _emit(tc, has_cout, has_bias):
    nc = tc.nc
    AF = mybir.ActivationFunctionType
    RED = bass_isa.ReduceOp

    xt_op = nc.dram_tensor("xt_op", [B, C, T], BF, kind="ExternalInput").ap()
    xt_att = nc.dram_tensor("xt_att", [B, C, T], BF, kind="ExternalInput").ap()
    wqT = nc.dram_tensor("wqT", [C, P], BF, kind="ExternalInput").ap()
    wkT = nc.dram_tensor("wkT", [C, P], BF, kind="ExternalInput").ap()
    wvT = nc.dram_tensor("wvT", [C, P], BF, kind="ExternalInput").ap()
    wpT = nc.dram_tensor("wpT", [P, O], BF, kind="ExternalInput").ap()
    qb = nc.dram_tensor("qb", [P, 1], F32, kind="ExternalInput").ap() if has_bias else None
    kb = nc.dram_tensor("kb", [P, 1], F32, kind="ExternalInput").ap() if has_bias else None
    coutb = (
        nc.dram_tensor("coutb", [128, O], F32, kind="ExternalInput").ap()
        if has_cout
        else None
    )
    out = nc.dram_tensor("out", [B, T, O], BF, kind="ExternalOutput").ap()

    with (
        tc.tile_pool(name="const", bufs=1) as const,
        tc.tile_pool(name="xt", bufs=2) as xtp,
        tc.tile_pool(name="qkv", bufs=2) as qkvp,
        tc.tile_pool(name="pt", bufs=2) as ptp,
        tc.tile_pool(name="tr", bufs=1) as trp,
        tc.tile_pool(name="svt", bufs=2) as svtp,
        tc.tile_pool(name="ost", bufs=2) as ostp,
        tc.tile_pool(name="ps", bufs=3, space="PSUM") as psp,
        tc.tile_pool(name="pv", bufs=1, space="PSUM") as pvp,
    ):
        # ---- constants / weights (wide tiles: contraction tile i at column
        # block i, so each tensor loads with ONE dma instruction) ----
        wq_w = const.tile([128, CT * P], BF, tag="wq", name="wq")
        wk_w = const.tile([128, CT * P], BF, tag="wk", name="wk")
        wv_w = const.tile([128, CT * P], BF, tag="wv", name="wv")
        wp_w = const.tile([128, CT * O], BF, tag="wp", name="wp")
        qb_w = const.tile([128, G], F32, tag="qb", name="qb") if has_bias else None
        kb_w = const.tile([128, G], F32, tag="kb", name="kb") if has_bias else None
        cout_s = const.tile([128, O], F32, tag="cout", name="cout") if has_cout else None
        ones_s = const.tile([128, 128], BF, tag="ones", name="ones")

        xo_s, xa_s = {}, {}
        for b in range(B):
            xo_s[b] = xtp.tile([128, CT * T], BF, tag="xo", name=f"xo_{b}")
            xa_s[b] = xtp.tile([128, CT * T], BF, tag="xa", name=f"xa_{b}")
        qT_s = {b: [qkvp.tile([128, T], BF, tag=f"q{g}", name=f"qT{g}_{b}") for g in range(G)] for b in range(B)}
        kT_s = {b: [qkvp.tile([128, T], BF, tag=f"k{g}", name=f"kT{g}_{b}") for g in range(G)] for b in range(B)}
        # v_s[b]: one tile [128, TT*512]; tile j lives at cols [j*512,(j+1)*512)
        # with partitions = the 128 t-rows of that j-tile.
        v_s = {b: qkvp.tile([128, TT * P], BF, tag="vv", name=f"v_{b}") for b in range(B)}
        svT = {b: [svtp.tile([128, T], BF, tag=f"s{g}", name=f"svT{g}_{b}") for g in range(G)] for b in range(B)}

        nc.vector.memset(ones_s[:], 1.0)

        # ---- input DMA, three parallel streams in per-queue need order ----
        def wide3(tile_ap, inner):
            return tile_ap.rearrange("p (ct t) -> p ct t", t=inner)

        def src3(dram2d, inner):
            return dram2d.rearrange("(ct p) t -> p ct t", p=128)

        def half(tile_ap, dram2d, inner, lo, hi):
            return wide3(tile_ap, inner)[:, lo:hi, :], src3(dram2d, inner)[:, lo:hi, :]

        def wgrp(tile_ap, dram2d, glo, ghi):
            # group-column slice [glo*128, ghi*128) of a [C, P] weight across
            # all 4 contraction blocks — unit (0,0) only needs group 0, so
            # the critical-path DMA shrinks to 128KB
            dst = wide3(tile_ap, P)[:, :, glo * 128 : ghi * 128]
            src = src3(dram2d, P)[:, :, glo * 128 : ghi * 128]
            return dst, src

        def mpiece(tile_ap, dram2d, ct0, mch):
            # {ct0, ct0+2} x one mch half: mch-major quarters so the first
            # q/k matmuls pipeline into the transfers AND each chunk's first
            # mch half arrives before its second
            dst = wide3(tile_ap, T)[:, ct0:4:2, mch * 512 : (mch + 1) * 512]
            src = src3(dram2d, T)[:, ct0:4:2, mch * 512 : (mch + 1) * 512]
            return dst, src

        # SP (HWDGE): even-ct pieces, mch0 before mch1, xo before xa
        nc.sync.dma_start(*mpiece(xo_s[0][:], xt_op[0], 0, 0))
        nc.sync.dma_start(*mpiece(xo_s[0][:], xt_op[0], 0, 1))
        nc.sync.dma_start(*mpiece(xa_s[0][:], xt_att[0], 0, 0))
        nc.sync.dma_start(*mpiece(xa_s[0][:], xt_att[0], 0, 1))
        nc.sync.dma_start(*half(wv_w[:], wvT, P, 0, 2))
        if has_cout:
            nc.sync.dma_start(cout_s[:], coutb[:, :])
        # ACT (HWDGE): group-0 weight columns first (before any ACT compute),
        # then the rest; all land well before exp0, and nothing later rides
        # this queue so the exp stream never waits behind a DMA instruction
        nc.scalar.dma_start(*wgrp(wq_w[:], wqT, 0, 1))
        nc.scalar.dma_start(*wgrp(wk_w[:], wkT, 0, 1))
        nc.scalar.dma_start(*wgrp(wq_w[:], wqT, 1, 4))
        nc.scalar.dma_start(*wgrp(wk_w[:], wkT, 1, 4))
        if has_bias:
            nc.scalar.dma_start(qb_w[:], qb.rearrange("(g p) o -> p (g o)", p=128))
            nc.scalar.dma_start(kb_w[:], kb.rearrange("(g p) o -> p (g o)", p=128))
        # Pool (SWDGE): odd ct-quarters + wv_h2, then the late tensors
        # (batch-1 activations + proj weights) in need order — they drain
        # here without ever touching the ACT queue
        nc.gpsimd.dma_start(*mpiece(xo_s[0][:], xt_op[0], 1, 0))
        nc.gpsimd.dma_start(*mpiece(xo_s[0][:], xt_op[0], 1, 1))
        nc.gpsimd.dma_start(*mpiece(xa_s[0][:], xt_att[0], 1, 0))
        nc.gpsimd.dma_start(*mpiece(xa_s[0][:], xt_att[0], 1, 1))
        nc.gpsimd.dma_start(*half(wv_w[:], wvT, P, 2, 4))
        nc.gpsimd.dma_start(wide3(xo_s[1][:], T), src3(xt_op[1], T))
        nc.gpsimd.dma_start(wide3(xa_s[1][:], T), src3(xt_att[1], T))
        nc.gpsimd.dma_start(wide3(wp_w[:], O), src3(wpT, O))

        # ---- PE warm-up during the DMA wait: raise activity so the DVS
        # controller ramps the PE clock before real matmuls start ----
        warm = psp.tile([128, 1024], F32, tag="ps", name="warm")
        for w in range(20):
            nc.tensor.matmul(
                warm[:, (w % 2) * 512 : (w % 2) * 512 + 128],
                ones_s[:],
                ones_s[:],
                start=True,
                stop=True,
            )
        # trigger the exp table load early (costs 2.7us once; without this it
        # lands right before the first real exp)
        warm_e = trp.tile([128, 1], F32, tag="we", name="warm_e")
        nc.scalar.activation(warm_e[:], ones_s[:, 0:1], AF.Exp)

        # ---- chunk emitters: one [128,1024] 2-bank PSUM pair + 1 fat evac ----
        def qk_evac(dst_ap, ps_ap, bias_w, g, ev):
            # ev 0 = DVE, 1 = ACT; bias add only when biases are nonzero
            if ev == 0:
                if has_bias:
                    nc.vector.tensor_scalar_add(dst_ap, ps_ap, bias_w[:, g : g + 1])
                else:
                    nc.vector.tensor_copy(dst_ap, ps_ap)
            else:
                if has_bias:
                    nc.scalar.activation(
                        dst_ap, ps_ap, AF.Identity, bias=bias_w[:, g : g + 1]
                    )
                else:
                    nc.scalar.copy(dst_ap, ps_ap)

        def chunk_qk_half(b, g, mch, kind, ev, pool_tag="ps"):
            # one 512-wide half of a q/k chunk in its own PSUM tile, so the
            # half evacuates while the other half's matmuls stream
            w_w, x_s, dst = (
                (wq_w, xo_s, qT_s) if kind == "q" else (wk_w, xa_s, kT_s)
            )
            pool = psp if pool_tag == "ps" else pvp
            ps = pool.tile(
                [128, 1024], F32, tag=pool_tag, name=f"ps{kind}{g}_{b}h{mch}"
            )
            for ct in range(CT):
                nc.tensor.matmul(
                    ps[:, 0:512],
                    w_w[:, ct * P + g * 128 : ct * P + (g + 1) * 128],
                    x_s[b][:, ct * T + mch * 512 : ct * T + (mch + 1) * 512],
                    start=(ct == 0),
                    stop=(ct == CT - 1),
                )
            qk_evac(
                dst[b][g][:, mch * 512 : (mch + 1) * 512],
                ps[:, 0:512],
                qb_w if kind == "q" else kb_w,
                g,
                ev,
            )

        def chunk_q(b, g, ev):
            ps = psp.tile([128, 1024], F32, tag="ps", name=f"psq{g}_{b}")
            for mch in range(MCH):
                for ct in range(CT):
                    nc.tensor.matmul(
                        ps[:, mch * 512 : (mch + 1) * 512],
                        wq_w[:, ct * P + g * 128 : ct * P + (g + 1) * 128],
                        xo_s[b][:, ct * T + mch * 512 : ct * T + (mch + 1) * 512],
                        start=(ct == 0),
                        stop=(ct == CT - 1),
                    )
            qk_evac(qT_s[b][g][:], ps[:], qb_w, g, ev)

        def chunk_k(b, g, ev):
            ps = psp.tile([128, 1024], F32, tag="ps", name=f"psk{g}_{b}")
            for mch in range(MCH):
                for ct in range(CT):
                    nc.tensor.matmul(
                        ps[:, mch * 512 : (mch + 1) * 512],
                        wk_w[:, ct * P + g * 128 : ct * P + (g + 1) * 128],
                        xa_s[b][:, ct * T + mch * 512 : ct * T + (mch + 1) * 512],
                        start=(ct == 0),
                        stop=(ct == CT - 1),
                    )
            qk_evac(kT_s[b][g][:], ps[:], kb_w, g, ev)

        def chunk_v(b, jj, ev, pool_tag="ps"):
            pool = psp if pool_tag == "ps" else pvp
            ps = pool.tile([128, 1024], F32, tag=pool_tag, name=f"psv{jj}_{b}")
            for jh in range(2):
                j = jj * 2 + jh
                for ct in range(CT):
                    nc.tensor.matmul(
                        ps[:, jh * 512 : (jh + 1) * 512],
                        xa_s[b][:, ct * T + j * 128 : ct * T + (j + 1) * 128],
                        wv_w[:, ct * P : (ct + 1) * P],
                        start=(ct == 0),
                        stop=(ct == CT - 1),
                    )
            osl = v_s[b][:, jj * 1024 : (jj + 1) * 1024]
            if ev == 0:
                nc.vector.tensor_copy(osl, ps[:])
            else:
                nc.scalar.copy(osl, ps[:])

        def chunk_proj(b, mt, oh, ev, split=False, tailq=False):
            ost = ostp.tile([128, T], BF, tag=f"ost{oh}", name=f"ost{mt}{oh}_{b}")
            odram = out[b, mt * 128 : (mt + 1) * 128, oh * 1024 : (oh + 1) * 1024]

            def evac(dst_sl, ps_ap, qeng, ev_):
                if has_cout:
                    nc.vector.tensor_add(
                        ost[:, dst_sl], ps_ap, cout_s[:, oh * 1024 : (oh + 1) * 1024][:, dst_sl]
                    )
                elif ev_ == 0:
                    nc.vector.tensor_copy(ost[:, dst_sl], ps_ap)
                else:
                    nc.scalar.copy(ost[:, dst_sl], ps_ap)
                qeng.dma_start(odram[:, dst_sl], ost[:, dst_sl])

            if split:
                # final chunk: per-half PSUM tiles so half 0's evac+DMA
                # overlaps half 1's matmuls (shorter drain tail); h0 on
                # ACT, h1 on DVE so the two evacs run in parallel, both
                # DMAs on the low-latency SP queue
                for oc in range(2):
                    ps = psp.tile([128, 1024], F32, tag="ps", name=f"pso{mt}{oh}_{b}h{oc}")
                    for g in range(G):
                        nc.tensor.matmul(
                            ps[:, 0:512],
                            svT[b][g][:, mt * 128 : (mt + 1) * 128],
                            wp_w[:, g * O + oh * 1024 + oc * 512 : g * O + oh * 1024 + (oc + 1) * 512],
                            start=(g == 0),
                            stop=(g == G - 1),
                        )
                    evac(
                        slice(oc * 512, (oc + 1) * 512),
                        ps[:, 0:512],
                        nc.gpsimd if oc == 0 else nc.sync,
                        1 if oc == 0 else 0,
                    )
                return
            ps = psp.tile([128, 1024], F32, tag="ps", name=f"pso{mt}{oh}_{b}")
            for oc in range(2):
                for g in range(G):
                    nc.tensor.matmul(
                        ps[:, oc * 512 : (oc + 1) * 512],
                        svT[b][g][:, mt * 128 : (mt + 1) * 128],
                        wp_w[:, g * O + oh * 1024 + oc * 512 : g * O + oh * 1024 + (oc + 1) * 512],
                        start=(g == 0),
                        stop=(g == G - 1),
                    )
            # tail chunks alternate SP/Pool so the final DMAs interleave
            # across both queues instead of serializing on one
            if tailq:
                oq = nc.sync if (mt + oh) % 2 == 1 else nc.gpsimd
            else:
                oq = nc.sync if (mt + oh) % 2 == 0 else nc.gpsimd
            evac(slice(0, 1024), ps[:], oq, ev)

        # ---- software-pipelined slots ----
        # Slot k runs unit k's score+exp phase (ACT-paced) overlapped with
        # unit k-1's SV matmuls + normalize tail (deps all ready), plus
        # woven filler chunks. PE never waits on the in-flight exp stream.
        ULIST = [(b, g) for b in range(B) for g in range(G)]
        ctx = {}

        def slot(k, fills, aux=()):
            fills = list(fills)
            aux = {pos: fn for pos, fn in aux}

            def fill():
                if fills:
                    f = fills.pop(0)
                    if f is not None:
                        f()

            uk = ULIST[k] if k < len(ULIST) else None
            cp = ctx.get(k - 1)
            c = None
            if uk is not None:
                b, g = uk
                c = {"u": uk}
                c["pT"] = [
                    ptp.tile([128, T], BF, tag=f"p{j}", name=f"pT{j}_{g}_{b}")
                    for j in range(TT)
                ]
                c["pv"] = pvp.tile([128, 1024], F32, tag="pv", name=f"pv_{g}_{b}")
                for tg in ("s01", "s23", "s45", "r03", "r05", "r06", "tsum", "svu"):
                    c[tg] = trp.tile([128, T], BF, tag=tg, name=f"{tg}_{g}_{b}")
                c["rden"] = trp.tile([128, T], F32, tag="rden", name=f"rden_{g}_{b}")
                ctx[k] = c

            def s(j):
                if c is not None:
                    b, g = uk
                    ps = psp.tile([128, 1024], F32, tag="ps", name=f"pss{j}_{g}_{b}")
                    for mch in range(MCH):
                        nc.tensor.matmul(
                            ps[:, mch * 512 : (mch + 1) * 512],
                            kT_s[b][g][:, j * 128 : (j + 1) * 128],
                            qT_s[b][g][:, mch * 512 : (mch + 1) * 512],
                            start=True,
                            stop=True,
                        )
                    nc.scalar.activation(c["pT"][j][:], ps[:], AF.Exp)
                if j in aux:
                    aux.pop(j)()

            def sv(j):
                if cp is None:
                    return
                pb, pg = cp["u"]
                for mch in range(MCH):
                    nc.tensor.matmul(
                        cp["pv"][:, mch * 512 : (mch + 1) * 512],
                        v_s[pb][:, j * 512 + pg * 128 : j * 512 + (pg + 1) * 128],
                        cp["pT"][j][:, mch * 512 : (mch + 1) * 512],
                        start=(j == 0),
                        stop=(j == TT - 1),
                    )

            def add(dst, a_, b_):
                if c is not None:
                    nc.vector.tensor_add(c[dst][:], a_, b_)

            s(0)
            sv(0)
            sv(1)
            s(1)
            sv(2)
            fill()
            s(2)
            sv(3)
            if c:
                add("s01", c["pT"][0][:], c["pT"][1][:])
            s(3)
            sv(4)
            fill()
            if c:
                add("s23", c["pT"][2][:], c["pT"][3][:])
                add("r03", c["s01"][:], c["s23"][:])
            s(4)
            sv(5)
            s(5)
            sv(6)
            fill()
            if c:
                add("s45", c["pT"][4][:], c["pT"][5][:])
                add("r05", c["r03"][:], c["s45"][:])
            s(6)
            sv(7)
            if cp is not None:
                # unnormalized SV out of PSUM (frees the accumulator pair),
                # then normalize with the reciprocal computed last slot
                pb, pg = cp["u"]
                nc.vector.tensor_copy(cp["svu"][:], cp["pv"][:])
                nc.vector.tensor_mul(svT[pb][pg][:], cp["svu"][:], cp["rden"][:])
            if c:
                add("r06", c["r05"][:], c["pT"][6][:])
            s(7)
            if c:
                # only ONE DVE add remains after the final exp
                add("tsum", c["r06"][:], c["pT"][7][:])
            # filler BETWEEN s(7) and the denominator matmuls: PE chews on it
            # while the exp7 -> r06 -> tsum DVE chain drains
            fill()
            if c:
                b, g = uk
                pd = psp.tile([128, 1024], F32, tag="ps", name=f"pd_{g}_{b}")
                for mch in range(MCH):
                    nc.tensor.matmul(
                        pd[:, mch * 512 : (mch + 1) * 512],
                        ones_s[:],
                        c["tsum"][:, mch * 512 : (mch + 1) * 512],
                        start=True,
                        stop=True,
                    )
                nc.vector.reciprocal_approx_fast(c["rden"][:], pd[:])
            while fills:
                f = fills.pop(0)
                if f is not None:
                    f()

        # ---- whole-kernel schedule ----
        def cq(b, g, ev):
            return lambda: chunk_q(b, g, ev)

        def ck(b, g, ev):
            return lambda: chunk_k(b, g, ev)

        def cv(b, jj, ev):
            return lambda: chunk_v(b, jj, ev)

        def fp(b, a, ev):
            return lambda: chunk_proj(b, a // 2, a % 2, ev)

        # prologue: the minimum for s(0)..s(3) — q halves + k's first half
        # (s(0..3) only read kT[:, 0:512]); k's second half rides slot 0 as
        # the first fill, in the SV-accumulator PSUM bank (free until slot 1)
        chunk_qk_half(0, 0, 0, "q", ev=0)
        chunk_qk_half(0, 0, 1, "q", ev=1)
        chunk_qk_half(0, 0, 0, "k", ev=0)
        # fill slots: F1/F2 evac on DVE (ev=0), F3+ on ACT (ev=1)
        # kh1 + cv(0,0) ride the pv buffer (free until slot 1's SV), giving
        # the PE non-psp-gated work while the exp chain paces psp recycling
        slot(0, [lambda: chunk_qk_half(0, 0, 1, "k", ev=0, pool_tag="pv"),
                 cv(0, 1, 1),
                 lambda: chunk_v(0, 0, 0, pool_tag="pv"),
                 cq(0, 1, 1), ck(0, 1, 1), cv(0, 2, 1)])
        slot(1, [cv(0, 3, 1), cq(0, 2, 1), ck(0, 2, 1), cq(1, 0, 1)])
        slot(2, [cq(0, 3, 1), ck(0, 3, 1), ck(1, 0, 1), cq(1, 1, 1)])
        slot(3, [ck(1, 1, 1), cv(1, 0, 1), cv(1, 1, 1), cv(1, 2, 1)])
        slot(4, [cv(1, 3, 1), cq(1, 2, 1), ck(1, 2, 1), cq(1, 3, 1)])
        # proj(0,*) needs svT(0,3), ready mid-slot 5: keep its early slots QKV
        slot(5, [ck(1, 3, 0), None, fp(0, 0, 1), fp(0, 1, 1)])
        slot(6, [fp(0, 2, 1), fp(0, 3, 1), fp(0, 4, 1), fp(0, 5, 1)])
        slot(7, [fp(0, 6, 1), fp(0, 7, 1), fp(0, 8, 1), fp(0, 9, 1)])
        slot(8, [fp(0, 10, 1), fp(0, 11, 1), fp(0, 12, 1), fp(0, 13, 1),
                 fp(0, 14, 1), fp(0, 15, 1)])
        # epilogue: batch-1 proj; last chunk split for a shorter drain tail
        for a in range(16):
            chunk_proj(1, a // 2, a % 2, ev=a % 2, split=(a == 15), tailq=(a >= 12))


def _build(has_cout, has_bias):
    nc = bacc.Bacc(
        "TRN2", target_bir_lowering=False, debug=False, num_devices=N_CORES
    )
    with tile.TileContext(nc) as tc:
        _emit(tc, has_cout, has_bias)
    nc.compile()
    return nc


def get_nc(has_cout=False, has_bias=False):
    key = ("nc", has_cout, has_bias)
    if key not in _CACHE:
        _CACHE[key] = _build(has_cout, has_bias)
    return _CACHE[key]


def prep_inputs(attention, op_param, q_w, q_b, k_w, k_b, v_w, v_b, proj_w, proj_b):
    """Host-side layout prep: fold scales, transpose, cast to bf16, shard."""
    bf16 = ml_dtypes.bfloat16
    f32 = np.float32

    att = np.asarray(attention, f32)
    op = np.asarray(op_param, f32)

    # (n, t, c) -> (n, c, t), bf16
    xt_att = np.ascontiguousarray(att.transpose(0, 2, 1)).astype(bf16)
    xt_op = np.ascontiguousarray(op.transpose(0, 2, 1)).astype(bf16)

    wqT = np.ascontiguousarray(
        (np.asarray(q_w, f32) * (EQ_SCALE * ATTN_SCALE)).T
    ).astype(bf16)
    wkT = np.ascontiguousarray((np.asarray(k_w, f32) * EQ_SCALE).T).astype(bf16)
    wvT = np.ascontiguousarray((np.asarray(v_w, f32) * EQ_SCALE).T).astype(bf16)
    wp_scaled = np.asarray(proj_w, f32) * EQ_SCALE
    wpT = np.ascontiguousarray(wp_scaled.T).astype(bf16)

    qb2 = (np.asarray(q_b, f32) * ATTN_SCALE).reshape(P, 1)
    kb2 = np.asarray(k_b, f32).reshape(P, 1)
    has_bias = bool(np.any(qb2 != 0.0) or np.any(kb2 != 0.0))
    # sum_l sim = 1, so v_b contributes proj_w @ v_b to every output row
    cout = wp_scaled @ np.asarray(v_b, f32) + np.asarray(proj_b, f32)
    has_cout = bool(np.any(cout != 0.0))
    coutb = np.ascontiguousarray(np.broadcast_to(cout[None, :], (128, O))).astype(f32)

    in_maps = []
    for core in range(N_CORES):
        lo, hi = core * B, (core + 1) * B
        m = {
            "xt_op": np.ascontiguousarray(xt_op[lo:hi]),
            "xt_att": np.ascontiguousarray(xt_att[lo:hi]),
            "wqT": wqT,
            "wkT": wkT,
            "wvT": wvT,
            "wpT": wpT,
        }
        if has_bias:
            m["qb"] = qb2
            m["kb"] = kb2
        if has_cout:
            m["coutb"] = coutb
        in_maps.append(m)
    return in_maps


def run(in_maps, trace=False, **kw):
    has_cout = "coutb" in in_maps[0]
    has_bias = "qb" in in_maps[0]
    nc = get_nc(has_cout, has_bias)
    res = run_bass_kernel_spmd(nc, in_maps, list(range(N_CORES)), trace=trace, **kw)
    return res


def kernel(**inputs) -> np.ndarray:
    in_maps = prep_inputs(**inputs)
    res = run(in_maps)
    out = np.concatenate(
        [np.asarray(res.results[i]["out"]) for i in range(N_CORES)], axis=0
    )
    return out.astype(np.float32)

